# revision 15
# baseline (speedup 1.0000x reference)
"""Causal self-attention (B=4, T=2048, D=1024, H=16) on 8 TRN2 NeuronCores.

Sharding: tensor-parallel over 4 head-groups x data-parallel over 2 batch-groups.
Core c handles batches [2*(c//4), 2*(c//4)+2) and heads [4*(c%4), 4*(c%4)+4).
Each core computes a partial output projection (its 256 feature rows of W_proj);
the host sums the 4 head-group partials per batch group.

v2: all operands bf16 (fp32 PSUM accumulation), which buys:
 - x^T produced by X-bar DMA transpose straight from HBM (no PE transposes,
   no untransposed x load at all).
 - S computed per head PAIR with two row-tiled K=64 matmuls (heads live on
   partitions 0:64 / 64:128 of qt/kt, tile_position (0,0)/(64,0)) writing the
   two 512-halves of one [128,1024] psS tile; on HW the two row groups run
   concurrently.
 - ONE exp activation per key tile covers both heads; the causal diagonal is
   handled by slicing the exp at the q-offset plus [128,128] triangle-mask
   multiplies on DVE.
 - PV uses a 65-column stationary (64 V cols + ones col, set once) so the
   softmax denominator accumulates in psY row 64 for free; division is
   DVE reciprocal -> gpsimd partition_broadcast -> DVE multiply (PSUM -> yt
   directly, no PE broadcast matmul, no PE stall).
Weights are DMA'd per-dk chunk on the scalar queue while the first x
transposes run on the sync queue, so the first matmul issues ~1.5us in.
The output projection for q-block j is emitted after block j+1's S/PV so the
PE never waits on the divide chain.
"""
import functools
from contextlib import ExitStack

import numpy as np
import ml_dtypes

import concourse.bacc as bacc
import concourse.tile as tile
import concourse.mybir as mybir
from concourse.bass_utils import run_bass_kernel_spmd

F32 = mybir.dt.float32
BF16 = mybir.dt.bfloat16
EXP = mybir.ActivationFunctionType.Exp

B, T, D, H, HD = 4, 2048, 1024, 16, 64
NB, NH = 2, 4            # batches / heads per core
DL = NH * HD             # local feature dim (256)
NC = 8
WCOL = 768               # per-dk weight columns: Q(256) K(256) V(256) packed
NT5 = T // 512           # 4  (512-token q blocks)
NTT = T // 128           # 16 (128-token key tiles)
NDK = D // 128           # 8  (feature chunks of input dim)


@functools.lru_cache(maxsize=1)
def build():
    nc = bacc.Bacc("TRN2", target_bir_lowering=False, debug=False, num_devices=NC)
    x_d = nc.dram_tensor("x", [NB, T, D], BF16, kind="ExternalInput").ap()
    wqkv_d = nc.dram_tensor("wqkv", [D, WCOL], BF16, kind="ExternalInput").ap()
    wproj_d = nc.dram_tensor("wproj", [DL, D], BF16, kind="ExternalInput").ap()
    tri_d = nc.dram_tensor("tri", [128, 128], BF16, kind="ExternalInput").ap()
    out_d = nc.dram_tensor("out", [NB, T, D], BF16, kind="ExternalOutput").ap()

    with tile.TileContext(nc) as tc, ExitStack() as ctx:
        const = ctx.enter_context(tc.tile_pool(name="const", bufs=1))
        wpool = ctx.enter_context(tc.tile_pool(name="w", bufs=1))
        xt_pool = ctx.enter_context(tc.tile_pool(name="xt", bufs=1))

        # First x^T transpose chunk at maximum priority: it serializes
        # against all other DMAs (deadlock guard), so let it run before the
        # weight chunks rather than in the middle of them.
        xts = [xt_pool.tile([128, NDK, T], BF16, tag=f"xt{bb}", name=f"xt{bb}")
               for bb in range(NB)]
        with tc.high_priority():
            nc.sync.dma_start_transpose(
                xts[0][:, :, 0:512], x_d[0, 0:512, :])

        tri = const.tile([128, 128], BF16)
        nc.scalar.dma_start(tri[:], tri_d)

        # weights: w_sb[:, dk*WCOL + c] = wqkv[dk*128 + p, c]; per-dk DMAs on
        # the scalar queue so the first Q matmul only waits for chunk 0.
        w_sb = wpool.tile([128, NDK * WCOL], BF16)
        for dk in range(NDK):
            nc.scalar.dma_start(
                w_sb[:, dk * WCOL:(dk + 1) * WCOL],
                wqkv_d[dk * 128:(dk + 1) * 128, :])
        wp_sb = wpool.tile([128, 2 * D], BF16)
        nc.scalar.dma_start(
            wp_sb[:].rearrange("p (a c) -> p a c", a=2),
            wproj_d.rearrange("(a p) c -> p a c", p=128))

        # V blocks per (key-tile ti, head h): 128 cols at (ti*NH+h)*128;
        # col 0 = ones (so the denominator lands in psY partition 0, where
        # reciprocal_approx_fast works - it breaks at base partition != 0),
        # cols 64:128 = V, cols 1:64 = zeros (psY rows 1:64 never read).
        v_sb = wpool.tile([128, NTT * NH * 128], BF16)
        v128 = v_sb[:].rearrange("p (n c) -> p n c", c=128)
        nc.gpsimd.memset(v_sb[:], 0.0)
        nc.gpsimd.memset(v128[:, :, 0:1], 1.0)

        # x^T via X-bar transpose: xt[p, dk, t] = x[b, t, dk*128+p].
        # One call per 512-token chunk keeps the DRAM side fully contiguous
        # (2KB rows). DMA_TRANSPOSE serializes against every other DMA
        # (deadlock guard), so all 8 transposes are emitted back-to-back
        # right after the small weight DMAs and before any output DMA.
        for b in range(NB):
            for t5 in range(NT5):
                if b == 0 and t5 == 0:
                    continue
                nc.sync.dma_start_transpose(
                    xts[b][:, :, 512 * t5:512 * (t5 + 1)],
                    x_d[b, 512 * t5:512 * (t5 + 1), :])

        for b in range(NB):
            xt = xts[b]
            with tc.tile_pool(name="actv", bufs=1) as actv:
                # Q^T / K^T packed per head pair hp: rows 0:64 head 2hp,
                # rows 64:128 head 2hp+1, full T columns.
                qt = [actv.tile([128, T], BF16, tag=f"qt{cc}", name=f"qt{cc}")
                      for cc in range(2)]
                kt = [actv.tile([128, T], BF16, tag=f"kt{cc}", name=f"kt{cc}")
                      for cc in range(2)]

                # ---- Phase A: Q^T, K^T, V ----
                with tc.tile_pool(name="psQK", bufs=2, space="PSUM") as psQK, \
                     tc.tile_pool(name="psV", bufs=2, space="PSUM") as psV:
                    for t5 in range(NT5):
                        ts = slice(512 * t5, 512 * (t5 + 1))
                        for cc in range(2):     # Q^T
                            ps = psQK.tile([128, 512], F32, tag="qk")
                            for dk in range(NDK):
                                nc.tensor.matmul(
                                    ps[:],
                                    w_sb[:, dk * WCOL + cc * 128:dk * WCOL + cc * 128 + 128],
                                    xt[:, dk, ts],
                                    start=(dk == 0), stop=(dk == NDK - 1))
                            nc.vector.tensor_copy(qt[cc][:, ts], ps[:])
                        for cc in range(2):     # K^T
                            ps = psQK.tile([128, 512], F32, tag="qk")
                            for dk in range(NDK):
                                nc.tensor.matmul(
                                    ps[:],
                                    w_sb[:, dk * WCOL + 256 + cc * 128:dk * WCOL + 256 + cc * 128 + 128],
                                    xt[:, dk, ts],
                                    start=(dk == 0), stop=(dk == NDK - 1))
                            nc.vector.tensor_copy(kt[cc][:, ts], ps[:])
                        for tt in range(4):     # V (tokens stationary)
                            ps = psV.tile([128, 256], F32, tag="v")
                            for dk in range(NDK):
                                nc.tensor.matmul(
                                    ps[:],
                                    xt[:, dk, 512 * t5 + 128 * tt:512 * t5 + 128 * tt + 128],
                                    w_sb[:, dk * WCOL + 512:dk * WCOL + 768],
                                    start=(dk == 0), stop=(dk == NDK - 1))
                            ti = t5 * 4 + tt
                            nc.vector.tensor_copy(
                                v128[:, ti * NH:(ti + 1) * NH, 64:128],
                                ps[:].rearrange("p (n c) -> p n c", c=64))

                # ---- Phase B: attention + projection ----
                with tc.tile_pool(name="psS", bufs=2, space="PSUM") as psS_pool, \
                     tc.tile_pool(name="psY", bufs=2, space="PSUM") as psY_pool, \
                     tc.tile_pool(name="psO", bufs=2, space="PSUM") as psO_pool, \
                     tc.tile_pool(name="pP", bufs=17) as pP, \
                     tc.tile_pool(name="ytp", bufs=2) as ytp, \
                     tc.tile_pool(name="rcp", bufs=2) as rcp, \
                     tc.tile_pool(name="ost", bufs=2) as ost_pool:

                    def proj_chunks(j, yts):
                        # output projection for q-block j as 8 emit-thunks
                        # (2 MMs + evac each); sprinkled into the next block's
                        # S-slots to fill exp-wait gaps on the PE.
                        state = {}

                        def chunk(g2, a, nn2):
                            def emit():
                                if g2 not in state:
                                    state[g2] = ost_pool.tile(
                                        [128, 2 * D], BF16, tag="o",
                                        name=f"ost{j}_{g2}")
                                ostage = state[g2]
                                tt = 2 * g2 + a
                                ps = psO_pool.tile([128, 512], F32, tag="o")
                                for ff in range(2):
                                    nc.tensor.matmul(
                                        ps[:],
                                        yts[ff][:, 128 * tt:128 * tt + 128],
                                        wp_sb[:, ff * D + 512 * nn2:ff * D + 512 * nn2 + 512],
                                        start=(ff == 0), stop=(ff == 1))
                                nc.vector.tensor_copy(
                                    ostage[:, a * D + 512 * nn2:a * D + 512 * nn2 + 512],
                                    ps[:])
                                if a == 1 and nn2 == 1:
                                    nc.sync.dma_start(
                                        out_d[b, 512 * j + 256 * g2:512 * j + 256 * g2 + 256]
                                        .rearrange("(a p) c -> p a c", p=128),
                                        ostage[:].rearrange("p (a c) -> p a c", a=2))
                            return emit

                        return [chunk(g2, a, nn2) for g2 in range(2)
                                for a in range(2) for nn2 in range(2)]

                    pending = []
                    for j in range(NT5):
                        yt = [ytp.tile([128, 512], BF16, tag=f"yt{ff}",
                                       name=f"yt{ff}") for ff in range(2)]
                        for hp in range(2):
                            qth, kth = qt[hp], kt[hp]
                            nk = 4 * j + 4
                            offs = [128 * (i - 4 * j) if i > 4 * j else 0
                                    for i in range(nk)]
                            Ps = []
                            for i in range(nk):
                                if pending:
                                    pending.pop(0)()
                                off = offs[i]
                                psS = psS_pool.tile([128, 1024], F32, tag="s")
                                P = pP.tile([128, 1024], BF16, tag="p")
                                Ps.append(P)
                                # two row-tiled K=64 matmuls (head pair)
                                nc.tensor.matmul(
                                    psS[:, off:512],
                                    kth[0:64, 128 * i:128 * i + 128],
                                    qth[0:64, 512 * j + off:512 * (j + 1)],
                                    start=True, stop=True)
                                nc.tensor.matmul(
                                    psS[:, 512 + off:1024],
                                    kth[64:128, 128 * i:128 * i + 128],
                                    qth[64:128, 512 * j + off:512 * (j + 1)],
                                    start=True, stop=True)
                                # one exp for both heads ([512+off-off:512] of
                                # the o-half below off is stale, never read)
                                nc.scalar.activation(
                                    P[:, off:1024], psS[:, off:1024], EXP,
                                    scale=0.125)
                                if i >= 4 * j:  # diagonal: causal triangle
                                    nc.vector.tensor_mul(
                                        P[:, off:off + 128],
                                        P[:, off:off + 128], tri[:])
                                    nc.vector.tensor_mul(
                                        P[:, 512 + off:512 + off + 128],
                                        P[:, 512 + off:512 + off + 128], tri[:])
                            for h01 in range(2):
                                h = 2 * hp + h01
                                psY = psY_pool.tile([128, 512], F32, tag="y")
                                for i in range(nk):
                                    off = offs[i]
                                    nc.tensor.matmul(
                                        psY[:, off:512],
                                        v_sb[:, 512 * i + 128 * h:512 * i + 128 * h + 128],
                                        Ps[i][:, h01 * 512 + off:h01 * 512 + 512],
                                        start=(i == 0), stop=(i == nk - 1))
                                # divide by the denominator (psY row 0)
                                rc = rcp.tile([1, 512], F32, tag="rc")
                                nc.vector.reciprocal_approx_fast(
                                    rc[:], psY[0:1, :])
                                rb = rcp.tile([128, 512], F32, tag="rb")
                                nc.gpsimd.partition_broadcast(rb[:], rc[:])
                                nc.vector.tensor_mul(
                                    yt[hp][64 * h01:64 * h01 + 64, :],
                                    psY[64:128, :], rb[64:128, :])
                        for fn in pending:   # any chunks not yet emitted
                            fn()
                        pending = proj_chunks(j, yt)
                    for fn in pending:
                        fn()

    nc.compile()
    return nc


def make_in_maps(x, W_qkv, W_proj):
    tri = np.triu(np.ones((128, 128), dtype=np.float32)).astype(ml_dtypes.bfloat16)
    in_maps = []
    for c in range(NC):
        bg, hg = c // 4, c % 4
        wq = np.concatenate(
            [W_qkv[:, 256 * hg:256 * hg + 256],
             W_qkv[:, 1024 + 256 * hg:1024 + 256 * hg + 256],
             W_qkv[:, 2048 + 256 * hg:2048 + 256 * hg + 256]], axis=1)
        in_maps.append({
            "x": np.ascontiguousarray(x[2 * bg:2 * bg + 2]).astype(ml_dtypes.bfloat16),
            "wqkv": wq.astype(ml_dtypes.bfloat16),
            "wproj": W_proj[256 * hg:256 * hg + 256, :].astype(ml_dtypes.bfloat16),
            "tri": tri,
        })
    return in_maps


def kernel(x, W_qkv, W_proj):
    x = np.asarray(x, dtype=np.float32)
    W_qkv = np.asarray(W_qkv, dtype=np.float32)
    W_proj = np.asarray(W_proj, dtype=np.float32)
    nc = build()
    res = run_bass_kernel_spmd(nc, make_in_maps(x, W_qkv, W_proj), list(range(NC)))
    out = np.zeros((B, T, D), dtype=np.float64)
    for c in range(NC):
        bg = c // 4
        out[2 * bg:2 * bg + 2] += res.results[c]["out"].astype(np.float64)
    return out.astype(np.float32)


# revision 17
# speedup vs baseline: 1.0658x; 1.0658x over previous
"""Causal self-attention (B=4, T=2048, D=1024, H=16) on 8 TRN2 NeuronCores.

Sharding: tensor-parallel over 4 head-groups x data-parallel over 2 batch-groups.
Core c handles batches [2*(c//4), 2*(c//4)+2) and heads [4*(c%4), 4*(c%4)+4).
Each core computes a partial output projection (its 256 feature rows of W_proj);
the host sums the 4 head-group partials per batch group.

v2: all operands bf16 (fp32 PSUM accumulation), which buys:
 - x^T produced by X-bar DMA transpose straight from HBM (no PE transposes,
   no untransposed x load at all).
 - S computed per head PAIR with two row-tiled K=64 matmuls (heads live on
   partitions 0:64 / 64:128 of qt/kt, tile_position (0,0)/(64,0)) writing the
   two 512-halves of one [128,1024] psS tile; on HW the two row groups run
   concurrently.
 - ONE exp activation per key tile covers both heads; the causal diagonal is
   handled by slicing the exp at the q-offset plus [128,128] triangle-mask
   multiplies on DVE.
 - PV uses a 65-column stationary (64 V cols + ones col, set once) so the
   softmax denominator accumulates in psY row 64 for free; division is
   DVE reciprocal -> gpsimd partition_broadcast -> DVE multiply (PSUM -> yt
   directly, no PE broadcast matmul, no PE stall).
Weights are DMA'd per-dk chunk on the scalar queue while the first x
transposes run on the sync queue, so the first matmul issues ~1.5us in.
The output projection for q-block j is emitted after block j+1's S/PV so the
PE never waits on the divide chain.
"""
import functools
from contextlib import ExitStack

import numpy as np
import ml_dtypes

import concourse.bacc as bacc
import concourse.tile as tile
import concourse.mybir as mybir
from concourse.bass_utils import run_bass_kernel_spmd

F32 = mybir.dt.float32
BF16 = mybir.dt.bfloat16
EXP = mybir.ActivationFunctionType.Exp

B, T, D, H, HD = 4, 2048, 1024, 16, 64
NB, NH = 2, 4            # batches / heads per core
DL = NH * HD             # local feature dim (256)
NC = 8
WCOL = 768               # per-dk weight columns: Q(256) K(256) V(256) packed
NT5 = T // 512           # 4  (512-token q blocks)
NTT = T // 128           # 16 (128-token key tiles)
NDK = D // 128           # 8  (feature chunks of input dim)


@functools.lru_cache(maxsize=1)
def build():
    nc = bacc.Bacc("TRN2", target_bir_lowering=False, debug=False, num_devices=NC)
    x_d = nc.dram_tensor("x", [NB, T, D], BF16, kind="ExternalInput").ap()
    wqkv_d = nc.dram_tensor("wqkv", [D, WCOL], BF16, kind="ExternalInput").ap()
    wproj_d = nc.dram_tensor("wproj", [DL, D], BF16, kind="ExternalInput").ap()
    tri_d = nc.dram_tensor("tri", [128, 128], BF16, kind="ExternalInput").ap()
    out_d = nc.dram_tensor("out", [NB, T, D], BF16, kind="ExternalOutput").ap()

    with tile.TileContext(nc) as tc, ExitStack() as ctx:
        const = ctx.enter_context(tc.tile_pool(name="const", bufs=1))
        wpool = ctx.enter_context(tc.tile_pool(name="w", bufs=1))
        xt_pool = ctx.enter_context(tc.tile_pool(name="xt", bufs=1))

        # Emission order = DMA serialization order (the transpose deadlock
        # guard serializes DMA_TRANSPOSE against every other DMA): first x^T
        # chunk, then the weight chunks, then the remaining transposes; tri
        # last (not needed until the first diagonal mask ~60us in).
        xts = [xt_pool.tile([128, NDK, T], BF16, tag=f"xt{bb}", name=f"xt{bb}")
               for bb in range(NB)]
        nc.sync.dma_start_transpose(xts[0][:, :, 0:512], x_d[0, 0:512, :])

        # weights: w_sb[:, dk*WCOL + c] = wqkv[dk*128 + p, c]; per-dk DMAs on
        # the scalar queue so the first Q matmul only waits for chunk 0.
        w_sb = wpool.tile([128, NDK * WCOL], BF16)
        for dk in range(NDK):
            nc.scalar.dma_start(
                w_sb[:, dk * WCOL:(dk + 1) * WCOL],
                wqkv_d[dk * 128:(dk + 1) * 128, :])
        wp_sb = wpool.tile([128, 2 * D], BF16)
        nc.scalar.dma_start(
            wp_sb[:].rearrange("p (a c) -> p a c", a=2),
            wproj_d.rearrange("(a p) c -> p a c", p=128))

        # V blocks per (key-tile ti, head h): 128 cols at (ti*NH+h)*128;
        # col 0 = ones (so the denominator lands in psY partition 0, where
        # reciprocal_approx_fast works - it breaks at base partition != 0),
        # cols 64:128 = V, cols 1:64 = zeros (psY rows 1:64 never read).
        v_sb = wpool.tile([128, NTT * NH * 128], BF16)
        v128 = v_sb[:].rearrange("p (n c) -> p n c", c=128)
        nc.gpsimd.memset(v_sb[:], 0.0)
        nc.gpsimd.memset(v128[:, :, 0:1], 1.0)

        # x^T via X-bar transpose: xt[p, dk, t] = x[b, t, dk*128+p].
        # One call per 512-token chunk keeps the DRAM side fully contiguous
        # (2KB rows). DMA_TRANSPOSE serializes against every other DMA
        # (deadlock guard), so all 8 transposes are emitted back-to-back
        # right after the small weight DMAs and before any output DMA.
        for b in range(NB):
            for t5 in range(NT5):
                if b == 0 and t5 == 0:
                    continue
                nc.sync.dma_start_transpose(
                    xts[b][:, :, 512 * t5:512 * (t5 + 1)],
                    x_d[b, 512 * t5:512 * (t5 + 1), :])

        tri = const.tile([128, 128], BF16)
        nc.scalar.dma_start(tri[:], tri_d)

        for b in range(NB):
            xt = xts[b]
            with tc.tile_pool(name="actv", bufs=1) as actv:
                # Q^T / K^T packed per head pair hp: rows 0:64 head 2hp,
                # rows 64:128 head 2hp+1, full T columns.
                qt = [actv.tile([128, T], BF16, tag=f"qt{cc}", name=f"qt{cc}")
                      for cc in range(2)]
                kt = [actv.tile([128, T], BF16, tag=f"kt{cc}", name=f"kt{cc}")
                      for cc in range(2)]

                # ---- Phase A: Q^T, K^T, V ----
                with tc.tile_pool(name="psQK", bufs=2, space="PSUM") as psQK, \
                     tc.tile_pool(name="psV", bufs=2, space="PSUM") as psV:
                    for t5 in range(NT5):
                        ts = slice(512 * t5, 512 * (t5 + 1))
                        for cc in range(2):     # Q^T
                            ps = psQK.tile([128, 512], F32, tag="qk")
                            for dk in range(NDK):
                                nc.tensor.matmul(
                                    ps[:],
                                    w_sb[:, dk * WCOL + cc * 128:dk * WCOL + cc * 128 + 128],
                                    xt[:, dk, ts],
                                    start=(dk == 0), stop=(dk == NDK - 1))
                            nc.vector.tensor_copy(qt[cc][:, ts], ps[:])
                        for cc in range(2):     # K^T
                            ps = psQK.tile([128, 512], F32, tag="qk")
                            for dk in range(NDK):
                                nc.tensor.matmul(
                                    ps[:],
                                    w_sb[:, dk * WCOL + 256 + cc * 128:dk * WCOL + 256 + cc * 128 + 128],
                                    xt[:, dk, ts],
                                    start=(dk == 0), stop=(dk == NDK - 1))
                            nc.vector.tensor_copy(kt[cc][:, ts], ps[:])
                        for tt in range(4):     # V (tokens stationary)
                            ps = psV.tile([128, 256], F32, tag="v")
                            for dk in range(NDK):
                                nc.tensor.matmul(
                                    ps[:],
                                    xt[:, dk, 512 * t5 + 128 * tt:512 * t5 + 128 * tt + 128],
                                    w_sb[:, dk * WCOL + 512:dk * WCOL + 768],
                                    start=(dk == 0), stop=(dk == NDK - 1))
                            ti = t5 * 4 + tt
                            nc.vector.tensor_copy(
                                v128[:, ti * NH:(ti + 1) * NH, 64:128],
                                ps[:].rearrange("p (n c) -> p n c", c=64))

                # ---- Phase B: attention + projection ----
                with tc.tile_pool(name="psS", bufs=2, space="PSUM") as psS_pool, \
                     tc.tile_pool(name="psY", bufs=2, space="PSUM") as psY_pool, \
                     tc.tile_pool(name="psO", bufs=2, space="PSUM") as psO_pool, \
                     tc.tile_pool(name="pP", bufs=17) as pP, \
                     tc.tile_pool(name="ytp", bufs=2) as ytp, \
                     tc.tile_pool(name="rcp", bufs=2) as rcp, \
                     tc.tile_pool(name="ost", bufs=2) as ost_pool:

                    def proj(j, yts):
                        # output projection for q-block j (yts from block j)
                        for g2 in range(2):
                            ostage = ost_pool.tile([128, 2 * D], BF16, tag="o")
                            for a in range(2):
                                tt = 2 * g2 + a
                                for nn2 in range(2):
                                    ps = psO_pool.tile([128, 512], F32, tag="o")
                                    for ff in range(2):
                                        nc.tensor.matmul(
                                            ps[:],
                                            yts[ff][:, 128 * tt:128 * tt + 128],
                                            wp_sb[:, ff * D + 512 * nn2:ff * D + 512 * nn2 + 512],
                                            start=(ff == 0), stop=(ff == 1))
                                    nc.vector.tensor_copy(
                                        ostage[:, a * D + 512 * nn2:a * D + 512 * nn2 + 512],
                                        ps[:])
                            nc.sync.dma_start(
                                out_d[b, 512 * j + 256 * g2:512 * j + 256 * g2 + 256]
                                .rearrange("(a p) c -> p a c", p=128),
                                ostage[:].rearrange("p (a c) -> p a c", a=2))

                    prev = None
                    for j in range(NT5):
                        yt = [ytp.tile([128, 512], BF16, tag=f"yt{ff}",
                                       name=f"yt{ff}") for ff in range(2)]
                        for hp in range(2):
                            qth, kth = qt[hp], kt[hp]
                            nk = 4 * j + 4
                            offs = [128 * (i - 4 * j) if i > 4 * j else 0
                                    for i in range(nk)]
                            Ps = []
                            for i in range(nk):
                                off = offs[i]
                                psS = psS_pool.tile([128, 1024], F32, tag="s")
                                P = pP.tile([128, 1024], BF16, tag="p")
                                Ps.append(P)
                                # two row-tiled K=64 matmuls (head pair)
                                nc.tensor.matmul(
                                    psS[:, off:512],
                                    kth[0:64, 128 * i:128 * i + 128],
                                    qth[0:64, 512 * j + off:512 * (j + 1)],
                                    start=True, stop=True)
                                nc.tensor.matmul(
                                    psS[:, 512 + off:1024],
                                    kth[64:128, 128 * i:128 * i + 128],
                                    qth[64:128, 512 * j + off:512 * (j + 1)],
                                    start=True, stop=True)
                                # one exp for both heads ([512+off-off:512] of
                                # the o-half below off is stale, never read)
                                nc.scalar.activation(
                                    P[:, off:1024], psS[:, off:1024], EXP,
                                    scale=0.125)
                                if i >= 4 * j:  # diagonal: causal triangle
                                    nc.vector.tensor_mul(
                                        P[:, off:off + 128],
                                        P[:, off:off + 128], tri[:])
                                    nc.vector.tensor_mul(
                                        P[:, 512 + off:512 + off + 128],
                                        P[:, 512 + off:512 + off + 128], tri[:])
                            for h01 in range(2):
                                h = 2 * hp + h01
                                psY = psY_pool.tile([128, 512], F32, tag="y")
                                for i in range(nk):
                                    off = offs[i]
                                    nc.tensor.matmul(
                                        psY[:, off:512],
                                        v_sb[:, 512 * i + 128 * h:512 * i + 128 * h + 128],
                                        Ps[i][:, h01 * 512 + off:h01 * 512 + 512],
                                        start=(i == 0), stop=(i == nk - 1))
                                # divide by the denominator (psY row 0)
                                rc = rcp.tile([1, 512], F32, tag="rc")
                                nc.vector.reciprocal_approx_fast(
                                    rc[:], psY[0:1, :])
                                rb = rcp.tile([128, 512], F32, tag="rb")
                                nc.gpsimd.partition_broadcast(rb[:], rc[:])
                                nc.vector.tensor_mul(
                                    yt[hp][64 * h01:64 * h01 + 64, :],
                                    psY[64:128, :], rb[64:128, :])
                        if prev is not None:
                            proj(*prev)
                        prev = (j, yt)
                    proj(*prev)

    nc.compile()
    return nc


def make_in_maps(x, W_qkv, W_proj):
    tri = np.triu(np.ones((128, 128), dtype=np.float32)).astype(ml_dtypes.bfloat16)
    in_maps = []
    for c in range(NC):
        bg, hg = c // 4, c % 4
        wq = np.concatenate(
            [W_qkv[:, 256 * hg:256 * hg + 256],
             W_qkv[:, 1024 + 256 * hg:1024 + 256 * hg + 256],
             W_qkv[:, 2048 + 256 * hg:2048 + 256 * hg + 256]], axis=1)
        in_maps.append({
            "x": np.ascontiguousarray(x[2 * bg:2 * bg + 2]).astype(ml_dtypes.bfloat16),
            "wqkv": wq.astype(ml_dtypes.bfloat16),
            "wproj": W_proj[256 * hg:256 * hg + 256, :].astype(ml_dtypes.bfloat16),
            "tri": tri,
        })
    return in_maps


def kernel(x, W_qkv, W_proj):
    x = np.asarray(x, dtype=np.float32)
    W_qkv = np.asarray(W_qkv, dtype=np.float32)
    W_proj = np.asarray(W_proj, dtype=np.float32)
    nc = build()
    res = run_bass_kernel_spmd(nc, make_in_maps(x, W_qkv, W_proj), list(range(NC)))
    out = np.zeros((B, T, D), dtype=np.float64)
    for c in range(NC):
        bg = c // 4
        out[2 * bg:2 * bg + 2] += res.results[c]["out"].astype(np.float64)
    return out.astype(np.float32)
